# revision 32
# baseline (speedup 1.0000x reference)
"""Trainium2 Bass kernel for GNN attention message passing — v2.

Reference computation (per query node b, step s, neighbors k=0..31):
    scores[s,b,k] = ne[s,b,k] . w_nb + node_e[b] . w_self + fc_b
    attn = softmax_k(leaky_relu(scores, 0.2))
    out[b] = sum_{s,k} attn[s,b,k] * ne[s,b,k] + S*K * node_e[b]

Sharding: data-parallel over the node batch B=4096 across 8 cores (512
query nodes per core).  Host-side prep lays the 32768 neighbor rows per
core out in two fp8 layouts so that BOTH heavy phases run as dense
DoubleRow (K=256) matmuls on the tensor engine:

  * NET8 [128, 16, 2, 2048]  d-on-partitions  -> score matmuls
    (stationary = w_nb replicated over 32 PE columns; 4 tile_position
    blocks spread the per-slot scores over all 128 PSUM partitions)
  * NE8R [128, 16, 8, 2, 256] slot-pairs-on-partitions -> aggregation
    (stationary = mask * attn, accumulated 8 pairs -> 32 query nodes)

fp8 storage is safe because the output is dominated by the (S*K)=64x
node_e term (kept in fp32); w_nb is prescaled by 16 and attn by 8 to
stay in fp8 e4m3's sweet spot, with exact power-of-two descales folded
into the softmax bias and the epilogue.

Slot order sigma = b*64 + s*32 + k. Score psum copies place slot sigma at
scores_sb[p, f] with p = 8*(sigma//2048) + (sigma%2048)//512 + 4*((sigma%512)//256),
f = sigma%256, so each partition holds 4 whole query nodes and softmax
runs in one [128, 4, 2, 32]-segmented pass on the vector engine.
"""

import os
import sys

for _p in ("/opt/trn_rl_repo", "/root/.axon_site/_ro/trn_rl_repo"):
    if os.path.isdir(_p) and _p not in sys.path:
        sys.path.insert(0, _p)

import numpy as np
import ml_dtypes

import concourse.bass as bass
import concourse.bacc as bacc
import concourse.tile as tile
from concourse import mybir
from concourse.bass_utils import run_bass_kernel_spmd

# Problem constants (hardcoded per spec)
N_NODES = 100000
D = 256
STEPS = 2
K = 32
B = 4096
NEG_SLOPE = 0.2
N_CORES = 8

B_LOC = B // N_CORES            # 512
SLOTS = B_LOC * STEPS * K       # 32768 (slot = b*64 + s*32 + k)
N_CHUNK = 16
CH_SLOTS = SLOTS // N_CHUNK     # 2048
N_PAIRS = SLOTS // 256          # 128 DoubleRow pairs
W_SCALE = 16.0                  # w_nb prescale for fp8 quantization
A_SCALE = 8.0                   # attn prescale (baked into the mask const)

F8 = np.dtype(ml_dtypes.float8_e4m3fn)
BF16 = np.dtype(ml_dtypes.bfloat16)

_CACHE = {}


def _build_nc(fc_w, fc_b, sim_safe=False):
    DT8 = mybir.dt.float8e4
    DTB = mybir.dt.bfloat16
    F32 = mybir.dt.float32

    nc = bacc.Bacc()

    net8_d = nc.dram_tensor("net8", [128, N_CHUNK, 2, CH_SLOTS], DT8,
                            kind="ExternalInput")
    ne8r_d = nc.dram_tensor("ne8r", [128, N_CHUNK, 8, 2, D], DT8,
                            kind="ExternalInput")
    node1_d = nc.dram_tensor("node1", [32, 16, D], F32, kind="ExternalInput")
    node2_d = nc.dram_tensor("node2", [4, 4, 32 * D], DTB, kind="ExternalInput")
    out_d = nc.dram_tensor("out", [B_LOC, D], F32, kind="ExternalOutput")

    w_nb = np.asarray(fc_w[0, :D], dtype=np.float32)
    w_self = np.asarray(fc_w[0, D:], dtype=np.float32)
    fcb = float(np.asarray(fc_b).reshape(-1)[0])

    # stationary for the score matmuls: w8dup[p, i, m] = q8(w_nb[128i+p] * 16)
    w16q = (w_nb * W_SCALE).astype(F8)
    w8dup_np = np.broadcast_to(
        w16q.reshape(2, 128).transpose(1, 0)[:, :, None], (128, 2, 32)
    ).copy()
    w8_c = nc.inline_tensor(w8dup_np, name="w8_c")

    # mask[q, pg, i, m] = 8.0 iff m == 4*pg + 2*i + q//64 (am = mask * attn)
    q = np.arange(128)
    mask_np = np.zeros((128, 8, 2, 32), dtype=np.float32)
    for pg in range(8):
        for i in range(2):
            mask_np[q, pg, i, 4 * pg + 2 * i + q // 64] = A_SCALE
    mask_c = nc.inline_tensor(mask_np.astype(F8), name="mask_c")

    ident_c = nc.inline_tensor(np.eye(128, dtype=np.float32).astype(BF16),
                               name="ident_c")
    wself_c = nc.inline_tensor(
        np.tile(w_self[None, :], (128, 1)).astype(BF16), name="wself_c"
    )

    DR = mybir.MatmulPerfMode.DoubleRow

    with tile.TileContext(nc) as tc:
        with (
            tc.tile_pool(name="consts", bufs=1) as consts,
            tc.tile_pool(name="netp", bufs=6) as netp,
            tc.tile_pool(name="nerp", bufs=4) as nerp,
            tc.tile_pool(name="smp", bufs=1) as smp,
            tc.tile_pool(name="amp", bufs=3) as amp,
            tc.tile_pool(name="outp", bufs=2) as outp,
            tc.tile_pool(name="scratch", bufs=6) as scratch,
            tc.tile_pool(name="psum_sc", bufs=4, space="PSUM") as psum_sc,
            tc.tile_pool(name="psum_t", bufs=2, space="PSUM") as psum_t,
            tc.tile_pool(name="psum_agg", bufs=2, space="PSUM") as psum_agg,
        ):
            # ---- constants (sync/scalar rings, ahead of the big streams) ----
            w8_sb = consts.tile([128, 2, 32], DT8, tag="w8")
            nc.sync.dma_start(out=w8_sb[:], in_=w8_c[:])
            mask_sb = consts.tile([128, 8, 2, 32], DT8, tag="mask")
            nc.scalar.dma_start(out=mask_sb[:], in_=mask_c[:])
            ident_sb = consts.tile([128, 128], DTB, tag="ident")
            nc.gpsimd.dma_start(out=ident_sb[:], in_=ident_c[:])
            wself_sb = consts.tile([128, D], DTB, tag="wself")
            nc.scalar.dma_start(out=wself_sb[:], in_=wself_c[:])
            node1_sb = consts.tile([32, 16, D], F32, tag="node1")
            node2_sb = consts.tile([128, 32, D], DTB, tag="node2")
            nc.vector.memset(node2_sb[:], 0)
            for P in range(4):
                nc.gpsimd.dma_start(
                    out=node2_sb[32 * P : 32 * P + 4, :, :], in_=node2_d[P]
                )

            # ---- c2[p, F, s8] = node_e(b) . w_self + fc_b  (16 valid parts)
            c2_sb = consts.tile([128, 32], F32, tag="c2_sb")

            _cstate = {}

            def emit_c_build(step):
                if step == 0:
                    _w = wself_sb[:]
                    wself_bc = bass.AP(tensor=_w.tensor, offset=_w.offset,
                                       ap=[_w.ap[0], [0, 32], [1, D]])
                    prodc = consts.tile([128, 32, D], DTB, tag="prodc")
                    _cstate["p"] = prodc
                    nc.vector.tensor_tensor(
                        out=prodc[:], in0=node2_sb[:], in1=wself_bc,
                        op=mybir.AluOpType.mult,
                    )
                elif step == 1:
                    prodc = _cstate["p"]
                    for w_half in (128, 64):
                        nc.vector.tensor_tensor(
                            out=prodc[:, :, 0:w_half],
                            in0=prodc[:, :, 0:w_half],
                            in1=prodc[:, :, w_half : 2 * w_half],
                            op=mybir.AluOpType.add,
                        )
                else:
                    prodc = _cstate["p"]
                    for w_half in (32, 16, 8, 4, 2, 1):
                        nc.vector.tensor_tensor(
                            out=prodc[:, :, 0:w_half],
                            in0=prodc[:, :, 0:w_half],
                            in1=prodc[:, :, w_half : 2 * w_half],
                            op=mybir.AluOpType.add,
                        )
                    nc.vector.tensor_scalar_add(
                        out=c2_sb[:], in0=prodc[:, :, 0], scalar1=fcb
                    )

            c2v = c2_sb[:].rearrange("p (F s8) -> p F s8", F=4)

            scores_sb = smp.tile([128, 4, 512], DTB, tag="scores")
            nc.scalar.memzero(scores_sb[:])
            nc.gpsimd.dma_start(out=node1_sb[:], in_=node1_d[:])
            ner_tiles = {}
            attnT = consts.tile([128, 256], DTB, tag="attnT")

            dr_kw = dict(perf_mode=DR, skip_group_check=True)

            def emit_softmax(F0, F1):
                nF = F1 - F0
                sv = scores_sb[:, F0:F1, :].rearrange(
                    "p F (s8 f) -> p F s8 f", s8=8)
                u = smp.tile([128, nF, 8, 64], DTB, tag="u", name=f"u{F0}")
                nc.vector.scalar_tensor_tensor(
                    out=u[:], in0=sv, scalar=1.0 / W_SCALE,
                    in1=c2v[:, F0:F1, :].to_broadcast([128, nF, 8, 64]),
                    op0=mybir.AluOpType.mult, op1=mybir.AluOpType.add,
                )
                uf = u[:].rearrange("p F s8 f -> p (F s8 f)")
                nfree = nF * 512
                lr = smp.tile([128, nfree], DTB, tag="lr", name=f"lr{F0}")
                nc.vector.scalar_tensor_tensor(
                    out=lr[:], in0=uf, scalar=NEG_SLOPE, in1=uf,
                    op0=mybir.AluOpType.mult, op1=mybir.AluOpType.max,
                )
                ex = smp.tile([128, nfree], DTB, tag="ex", name=f"ex{F0}")
                nc.scalar.activation(
                    out=ex[:], in_=lr[:],
                    func=mybir.ActivationFunctionType.Exp
                )
                nt = nfree // 32
                exv = ex[:].rearrange("p (t k) -> p t k", t=nt)
                ef = smp.tile([128, nt, 16], DTB, tag="ef", name=f"ef{F0}")
                nc.vector.tensor_tensor(
                    out=ef[:], in0=exv[:, :, 0:16], in1=exv[:, :, 16:32],
                    op=mybir.AluOpType.add,
                )
                w_half = 8
                while w_half >= 1:
                    nc.vector.tensor_tensor(
                        out=ef[:, :, 0:w_half],
                        in0=ef[:, :, 0:w_half],
                        in1=ef[:, :, w_half : 2 * w_half],
                        op=mybir.AluOpType.add,
                    )
                    w_half //= 2
                rcp = smp.tile([128, nt], F32, tag="rcp", name=f"rcp{F0}")
                nc.vector.reciprocal(out=rcp[:], in_=ef[:, :, 0])
                attn = smp.tile([128, nfree], DTB, tag="attn",
                                name=f"attn{F0}")
                nc.vector.tensor_tensor(
                    out=attn[:].rearrange("p (t k) -> p t k", t=nt),
                    in0=exv,
                    in1=rcp[:].to_broadcast([128, nt, 32]),
                    op=mybir.AluOpType.mult,
                )
                for tl in range(4 * nF):
                    t = 4 * F0 + tl
                    t_ps = psum_t.tile([128, 128], DTB, tag="t_ps")
                    nc.tensor.transpose(
                        out=t_ps[:],
                        in_=attn[:, 128 * tl : 128 * (tl + 1)],
                        identity=ident_sb[:],
                    )
                    tsrc = t_ps[:]
                    in_tp = bass.AP(
                        tensor=tsrc.tensor, offset=tsrc.offset,
                        ap=[tsrc.ap[0], [32, 4], [4, 4]],
                    )
                    adst = attnT[:]
                    out_tp = bass.AP(
                        tensor=adst.tensor,
                        offset=adst.offset + (t % 4) + 64 * (t // 4),
                        ap=[adst.ap[0], [16, 4], [4, 4]],
                    )
                    nc.vector.tensor_copy(out=out_tp, in_=in_tp)


            # ---- phase 2: am + aggregation + epilogue per group ----
            am_tiles = {}
            _done_groups = set()

            def emit_group(g):
                am = am_tiles.pop(g)
                ner_sb = ner_tiles[g // 4][:, g % 4]
                agg = psum_agg.tile([32, D], F32, tag="agg")
                for j in range(8):
                    nc.tensor.matmul(
                        out=agg[:],
                        lhsT=am[:, j, :, :],
                        rhs=ner_sb[:, j, :, :],
                        start=(j == 0), stop=(j == 7), **dr_kw,
                    )
                o_sb = outp.tile([32, D], F32, tag="o_sb")
                nc.vector.scalar_tensor_tensor(
                    out=o_sb[:], in0=agg[:], scalar=1.0 / A_SCALE,
                    in1=node1_sb[:, g, :],
                    op0=mybir.AluOpType.mult, op1=mybir.AluOpType.add,
                )
                (nc.sync if g % 2 == 0 else nc.scalar).dma_start(
                    out=out_d[32 * g : 32 * (g + 1), :], in_=o_sb[:]
                )
                _done_groups.add(g)

            def emit_am(g):
                am = amp.tile([128, 8, 2, 32], DT8, tag="am", name=f"am{g}")
                a_src = attnT[:]
                attn_bc = bass.AP(
                    tensor=a_src.tensor,
                    offset=a_src.offset + 16 * g,
                    ap=[a_src.ap[0], [2, 8], [1, 2], [0, 32]],
                )
                nc.vector.tensor_tensor(
                    out=am[:], in0=mask_sb[:], in1=attn_bc,
                    op=mybir.AluOpType.mult,
                )
                am_tiles[g] = am


            # ---- phase 1: stream NET8 (supers), score matmuls, evac ----
            dr_kw = dict(perf_mode=DR, skip_group_check=True)
            net_tiles = {}
            for c in range(N_CHUNK):
                net_sb_t = netp.tile([128, 2, CH_SLOTS], DT8,
                                     name=f"net{c}", tag="net")
                (nc.sync if c % 2 == 0 else nc.scalar).dma_start(
                    out=net_sb_t[:], in_=net8_d[:, c, :, :])
                if c == 12:
                    for SF in range(4):
                        ner_sb4 = nerp.tile([128, 4, 8, 2, D], DT8,
                                            name=f"ner{SF}", tag="ner")
                        (nc.scalar if SF % 2 == 0 else nc.sync).dma_start(
                            out=ner_sb4[:],
                            in_=ne8r_d[:, 4 * SF : 4 * SF + 4, :, :],
                        )
                        ner_tiles[SF] = ner_sb4
                net_sb = net_sb_t[:]
                sc_ps = psum_sc.tile([128, 512], F32, tag="sc_ps")
                for m in range(4):
                    for i in range(2):
                        nc.tensor.matmul(
                            out=sc_ps[32 * m : 32 * m + 32, :],
                            lhsT=w8_sb[:, i, :],
                            rhs=net_sb[:, i, 512 * m : 512 * (m + 1)],
                            start=(i == 0),
                            stop=(i == 1),
                            tile_position=(0, 32 * m),
                            skip_group_check=True,
                        )
                scdup = scratch.tile([128, 512], DTB, name=f"scd{c}",
                                     tag="scdup")
                if c % 2 == 0:
                    nc.vector.tensor_copy(out=scdup[:], in_=sc_ps[:])
                else:
                    nc.scalar.copy(out=scdup[:], in_=sc_ps[:])
                sd = scdup[:]
                in_ap = bass.AP(
                    tensor=sd.tensor,
                    offset=sd.offset,
                    ap=[[32 * sd.ap[0][0], 4]] + list(sd.ap[1:]),
                )
                nc.gpsimd.dma_start(
                    out=scores_sb[32 * (c % 4) : 32 * (c % 4) + 4, c // 4, :],
                    in_=in_ap,
                )
                if c in (3, 5, 6):
                    emit_c_build((3, 5, 6).index(c))
                if c == 11:
                    emit_softmax(0, 2)
                elif c == 13:
                    emit_am(0)
                    emit_group(0)
                elif c == 14:
                    emit_softmax(2, 3)
                    emit_am(1)
                    emit_group(1)
                elif c == 15:
                    emit_am(2)
                    emit_group(2)
                    emit_am(3)
                    emit_group(3)


            for g in range(4, 12):
                if g not in am_tiles and g not in _done_groups:
                    emit_am(g)
                if g + 1 < 12 and g + 1 not in am_tiles \
                        and g + 1 not in _done_groups:
                    emit_am(g + 1)
                if g not in _done_groups:
                    emit_group(g)
            emit_softmax(3, 4)

            # ---- phase 2 driver ----
            for g in range(N_CHUNK):
                if g not in am_tiles and g not in _done_groups:
                    emit_am(g)
                if g + 1 < N_CHUNK and g + 1 not in am_tiles \
                        and g + 1 not in _done_groups:
                    emit_am(g + 1)
                if g not in _done_groups:
                    emit_group(g)

    nc.compile()
    return nc


def _prep_core_inputs(core, node, neighbors, emb8, emb_f32):
    """Host-side sharding: pregather this core's neighbor rows into the
    two fp8 layouts plus the fp32/bf16 node tables."""
    node_c = np.asarray(node[B_LOC * core : B_LOC * (core + 1)])
    nb = np.asarray(neighbors[:, node_c, :])          # [S, B_LOC, K]
    flat = nb.transpose(1, 0, 2).reshape(-1)          # slot = b*64 + s*32 + k
    rows8 = emb8[flat]                                # [SLOTS, D] fp8

    # NET8[p, c, i, f] = rows8[2048c + f, 128i + p]
    net8 = np.ascontiguousarray(
        rows8.view(np.uint8).reshape(N_CHUNK, CH_SLOTS, 2, 128)
        .transpose(3, 0, 2, 1)
    ).view(F8)
    # NE8R[p, c, j, i, d] = rows8[2048c + 256j + 128i + p, d]
    ne8r = np.ascontiguousarray(
        rows8.view(np.uint8).reshape(N_CHUNK, 8, 2, 128, D)
        .transpose(3, 0, 1, 2, 4)
    ).view(F8)

    nd = emb_f32[node_c]                              # [B_LOC, D] f32
    # node1[q, g, :] = 64 * node_e[32g + q]
    node1 = np.ascontiguousarray(
        (nd * float(STEPS * K)).reshape(16, 32, D).transpose(1, 0, 2)
    )
    # node2[P, m, F, s8, :] = node_e[b], b = 128F + 32P + 8m + s8  (bf16)
    P = np.arange(4)[:, None, None, None]
    m = np.arange(4)[None, :, None, None]
    F = np.arange(4)[None, None, :, None]
    s8 = np.arange(8)[None, None, None, :]
    bmap = 128 * F + 32 * P + 8 * m + s8            # [4, 4, 4, 8]
    node2 = np.ascontiguousarray(
        nd[bmap].astype(BF16).reshape(4, 4, 32 * D)
    )

    return {"net8": net8, "ne8r": ne8r, "node1": node1, "node2": node2}


def kernel(node, neighbors, embeddings, fc_w, fc_b, _trace=False):
    node = np.asarray(node)
    neighbors = np.asarray(neighbors)
    emb_f32 = np.asarray(embeddings, dtype=np.float32)
    fc_w = np.asarray(fc_w, dtype=np.float32)
    fc_b = np.asarray(fc_b, dtype=np.float32)

    key = (fc_w.tobytes(), fc_b.tobytes())
    if _CACHE.get("key") != key:
        _CACHE["nc"] = _build_nc(fc_w, fc_b)
        _CACHE["key"] = key
    nc = _CACHE["nc"]

    emb8 = emb_f32.astype(F8)
    in_maps = [
        _prep_core_inputs(c, node, neighbors, emb8, emb_f32)
        for c in range(N_CORES)
    ]
    res = run_bass_kernel_spmd(
        nc, in_maps, core_ids=list(range(N_CORES)), trace=_trace
    )
    out = np.concatenate([res.results[c]["out"] for c in range(N_CORES)], axis=0)
    if _trace:
        _CACHE["last_exec_time_ns"] = res.exec_time_ns
        _CACHE["last_results"] = res
    return out


# revision 34
# speedup vs baseline: 1.0216x; 1.0216x over previous
"""Trainium2 Bass kernel for GNN attention message passing — v2.

Reference computation (per query node b, step s, neighbors k=0..31):
    scores[s,b,k] = ne[s,b,k] . w_nb + node_e[b] . w_self + fc_b
    attn = softmax_k(leaky_relu(scores, 0.2))
    out[b] = sum_{s,k} attn[s,b,k] * ne[s,b,k] + S*K * node_e[b]

Sharding: data-parallel over the node batch B=4096 across 8 cores (512
query nodes per core).  Host-side prep lays the 32768 neighbor rows per
core out in two fp8 layouts so that BOTH heavy phases run as dense
DoubleRow (K=256) matmuls on the tensor engine:

  * NET8 [128, 16, 2, 2048]  d-on-partitions  -> score matmuls
    (stationary = w_nb replicated over 32 PE columns; 4 tile_position
    blocks spread the per-slot scores over all 128 PSUM partitions)
  * NE8R [128, 16, 8, 2, 256] slot-pairs-on-partitions -> aggregation
    (stationary = mask * attn, accumulated 8 pairs -> 32 query nodes)

fp8 storage is safe because the output is dominated by the (S*K)=64x
node_e term (kept in fp32); w_nb is prescaled by 16 and attn by 8 to
stay in fp8 e4m3's sweet spot, with exact power-of-two descales folded
into the softmax bias and the epilogue.

Slot order sigma = b*64 + s*32 + k. Score psum copies place slot sigma at
scores_sb[p, f] with p = 8*(sigma//2048) + (sigma%2048)//512 + 4*((sigma%512)//256),
f = sigma%256, so each partition holds 4 whole query nodes and softmax
runs in one [128, 4, 2, 32]-segmented pass on the vector engine.
"""

import os
import sys

for _p in ("/opt/trn_rl_repo", "/root/.axon_site/_ro/trn_rl_repo"):
    if os.path.isdir(_p) and _p not in sys.path:
        sys.path.insert(0, _p)

import numpy as np
import ml_dtypes

import concourse.bass as bass
import concourse.bacc as bacc
import concourse.tile as tile
from concourse import mybir
from concourse.bass_utils import run_bass_kernel_spmd

# Problem constants (hardcoded per spec)
N_NODES = 100000
D = 256
STEPS = 2
K = 32
B = 4096
NEG_SLOPE = 0.2
N_CORES = 8

B_LOC = B // N_CORES            # 512
SLOTS = B_LOC * STEPS * K       # 32768 (slot = b*64 + s*32 + k)
N_CHUNK = 16
CH_SLOTS = SLOTS // N_CHUNK     # 2048
N_PAIRS = SLOTS // 256          # 128 DoubleRow pairs
W_SCALE = 16.0                  # w_nb prescale for fp8 quantization
A_SCALE = 8.0                   # attn prescale (baked into the mask const)

F8 = np.dtype(ml_dtypes.float8_e4m3fn)
BF16 = np.dtype(ml_dtypes.bfloat16)

_CACHE = {}


def _build_nc(fc_w, fc_b, sim_safe=False):
    DT8 = mybir.dt.float8e4
    DTB = mybir.dt.bfloat16
    F32 = mybir.dt.float32

    nc = bacc.Bacc()

    net8_d = nc.dram_tensor("net8", [128, N_CHUNK, 2, CH_SLOTS], DT8,
                            kind="ExternalInput")
    ne8r_d = nc.dram_tensor("ne8r", [128, N_CHUNK, 8, 2, D], DT8,
                            kind="ExternalInput")
    node1_d = nc.dram_tensor("node1", [32, 16, D], F32, kind="ExternalInput")
    node2_d = nc.dram_tensor("node2", [4, 4, 32 * D], DTB, kind="ExternalInput")
    out_d = nc.dram_tensor("out", [B_LOC, D], F32, kind="ExternalOutput")

    w_nb = np.asarray(fc_w[0, :D], dtype=np.float32)
    w_self = np.asarray(fc_w[0, D:], dtype=np.float32)
    fcb = float(np.asarray(fc_b).reshape(-1)[0])

    # stationary for the score matmuls: w8dup[p, i, m] = q8(w_nb[128i+p] * 16)
    w16q = (w_nb * W_SCALE).astype(F8)
    w8dup_np = np.broadcast_to(
        w16q.reshape(2, 128).transpose(1, 0)[:, :, None], (128, 2, 32)
    ).copy()
    w8_c = nc.inline_tensor(w8dup_np, name="w8_c")

    # mask[q, pg, i, m] = 8.0 iff m == 4*pg + 2*i + q//64 (am = mask * attn)
    q = np.arange(128)
    mask_np = np.zeros((128, 8, 2, 32), dtype=np.float32)
    for pg in range(8):
        for i in range(2):
            mask_np[q, pg, i, 4 * pg + 2 * i + q // 64] = A_SCALE
    mask_c = nc.inline_tensor(mask_np.astype(F8), name="mask_c")

    ident_c = nc.inline_tensor(np.eye(128, dtype=np.float32).astype(BF16),
                               name="ident_c")
    wself_c = nc.inline_tensor(
        np.tile(w_self[None, :], (128, 1)).astype(BF16), name="wself_c"
    )

    DR = mybir.MatmulPerfMode.DoubleRow

    with tile.TileContext(nc) as tc:
        with (
            tc.tile_pool(name="consts", bufs=1) as consts,
            tc.tile_pool(name="netp", bufs=4) as netp,
            tc.tile_pool(name="nerp", bufs=4) as nerp,
            tc.tile_pool(name="smp", bufs=1) as smp,
            tc.tile_pool(name="amp", bufs=3) as amp,
            tc.tile_pool(name="outp", bufs=2) as outp,
            tc.tile_pool(name="scratch", bufs=8) as scratch,
            tc.tile_pool(name="psum_sc", bufs=5, space="PSUM") as psum_sc,
            tc.tile_pool(name="psum_t", bufs=1, space="PSUM") as psum_t,
            tc.tile_pool(name="psum_agg", bufs=2, space="PSUM") as psum_agg,
        ):
            # ---- constants (sync/scalar rings, ahead of the big streams) ----
            w8_sb = consts.tile([128, 2, 32], DT8, tag="w8")
            nc.sync.dma_start(out=w8_sb[:], in_=w8_c[:])
            mask_sb = consts.tile([128, 8, 2, 32], DT8, tag="mask")
            nc.scalar.dma_start(out=mask_sb[:], in_=mask_c[:])
            ident_sb = consts.tile([128, 128], DTB, tag="ident")
            nc.gpsimd.dma_start(out=ident_sb[:], in_=ident_c[:])
            wself_sb = consts.tile([128, D], DTB, tag="wself")
            nc.scalar.dma_start(out=wself_sb[:], in_=wself_c[:])
            node1_sb = consts.tile([32, 16, D], F32, tag="node1")
            node2_sb = consts.tile([128, 32, D], DTB, tag="node2")
            nc.vector.memset(node2_sb[:], 0)
            for P in range(4):
                nc.gpsimd.dma_start(
                    out=node2_sb[32 * P : 32 * P + 4, :, :], in_=node2_d[P]
                )

            # ---- c2[p, F, s8] = node_e(b) . w_self + fc_b  (16 valid parts)
            c2_sb = consts.tile([128, 32], F32, tag="c2_sb")

            _cstate = {}

            def emit_c_build(step):
                if step == 0:
                    _w = wself_sb[:]
                    wself_bc = bass.AP(tensor=_w.tensor, offset=_w.offset,
                                       ap=[_w.ap[0], [0, 32], [1, D]])
                    prodc = consts.tile([128, 32, D], DTB, tag="prodc")
                    _cstate["p"] = prodc
                    nc.vector.tensor_tensor(
                        out=prodc[:], in0=node2_sb[:], in1=wself_bc,
                        op=mybir.AluOpType.mult,
                    )
                elif step == 1:
                    prodc = _cstate["p"]
                    for w_half in (128, 64):
                        nc.vector.tensor_tensor(
                            out=prodc[:, :, 0:w_half],
                            in0=prodc[:, :, 0:w_half],
                            in1=prodc[:, :, w_half : 2 * w_half],
                            op=mybir.AluOpType.add,
                        )
                else:
                    prodc = _cstate["p"]
                    for w_half in (32, 16, 8, 4, 2, 1):
                        nc.vector.tensor_tensor(
                            out=prodc[:, :, 0:w_half],
                            in0=prodc[:, :, 0:w_half],
                            in1=prodc[:, :, w_half : 2 * w_half],
                            op=mybir.AluOpType.add,
                        )
                    nc.vector.tensor_scalar_add(
                        out=c2_sb[:], in0=prodc[:, :, 0], scalar1=fcb
                    )

            c2v = c2_sb[:].rearrange("p (F s8) -> p F s8", F=4)

            scores_sb = smp.tile([128, 4, 512], DTB, tag="scores")
            nc.scalar.memzero(scores_sb[:])
            nc.gpsimd.dma_start(out=node1_sb[:], in_=node1_d[:])
            ner_tiles = {}
            attnT = consts.tile([128, 256], DTB, tag="attnT")

            dr_kw = dict(perf_mode=DR, skip_group_check=True)

            def emit_softmax(F0, F1):
                nF = F1 - F0
                sv = scores_sb[:, F0:F1, :].rearrange(
                    "p F (s8 f) -> p F s8 f", s8=8)
                u = smp.tile([128, nF, 8, 64], DTB, tag="u", name=f"u{F0}")
                nc.vector.scalar_tensor_tensor(
                    out=u[:], in0=sv, scalar=1.0 / W_SCALE,
                    in1=c2v[:, F0:F1, :].to_broadcast([128, nF, 8, 64]),
                    op0=mybir.AluOpType.mult, op1=mybir.AluOpType.add,
                )
                uf = u[:].rearrange("p F s8 f -> p (F s8 f)")
                nfree = nF * 512
                lr = smp.tile([128, nfree], DTB, tag="lr", name=f"lr{F0}")
                nc.vector.scalar_tensor_tensor(
                    out=lr[:], in0=uf, scalar=NEG_SLOPE, in1=uf,
                    op0=mybir.AluOpType.mult, op1=mybir.AluOpType.max,
                )
                ex = smp.tile([128, nfree], DTB, tag="ex", name=f"ex{F0}")
                nc.scalar.activation(
                    out=ex[:], in_=lr[:],
                    func=mybir.ActivationFunctionType.Exp
                )
                nt = nfree // 32
                exv = ex[:].rearrange("p (t k) -> p t k", t=nt)
                ef = smp.tile([128, nt, 16], DTB, tag="ef", name=f"ef{F0}")
                nc.vector.tensor_tensor(
                    out=ef[:], in0=exv[:, :, 0:16], in1=exv[:, :, 16:32],
                    op=mybir.AluOpType.add,
                )
                w_half = 8
                while w_half >= 1:
                    nc.vector.tensor_tensor(
                        out=ef[:, :, 0:w_half],
                        in0=ef[:, :, 0:w_half],
                        in1=ef[:, :, w_half : 2 * w_half],
                        op=mybir.AluOpType.add,
                    )
                    w_half //= 2
                rcp = smp.tile([128, nt], F32, tag="rcp", name=f"rcp{F0}")
                nc.vector.reciprocal(out=rcp[:], in_=ef[:, :, 0])
                attn = smp.tile([128, nfree], DTB, tag="attn",
                                name=f"attn{F0}")
                nc.vector.tensor_tensor(
                    out=attn[:].rearrange("p (t k) -> p t k", t=nt),
                    in0=exv,
                    in1=rcp[:].to_broadcast([128, nt, 32]),
                    op=mybir.AluOpType.mult,
                )
                for tl in range(4 * nF):
                    t = 4 * F0 + tl
                    t_ps = psum_t.tile([128, 128], DTB, tag="t_ps")
                    nc.tensor.transpose(
                        out=t_ps[:],
                        in_=attn[:, 128 * tl : 128 * (tl + 1)],
                        identity=ident_sb[:],
                    )
                    tsrc = t_ps[:]
                    in_tp = bass.AP(
                        tensor=tsrc.tensor, offset=tsrc.offset,
                        ap=[tsrc.ap[0], [32, 4], [4, 4]],
                    )
                    adst = attnT[:]
                    out_tp = bass.AP(
                        tensor=adst.tensor,
                        offset=adst.offset + (t % 4) + 64 * (t // 4),
                        ap=[adst.ap[0], [16, 4], [4, 4]],
                    )
                    nc.vector.tensor_copy(out=out_tp, in_=in_tp)


            # ---- phase 2: am + aggregation + epilogue per group ----
            am_tiles = {}
            _done_groups = set()

            def emit_group(g):
                am = am_tiles.pop(g)
                ner_sb = ner_tiles[g // 4][:, g % 4]
                agg = psum_agg.tile([32, D], F32, tag="agg")
                for j in range(8):
                    nc.tensor.matmul(
                        out=agg[:],
                        lhsT=am[:, j, :, :],
                        rhs=ner_sb[:, j, :, :],
                        start=(j == 0), stop=(j == 7), **dr_kw,
                    )
                o_sb = outp.tile([32, D], F32, tag="o_sb")
                nc.vector.scalar_tensor_tensor(
                    out=o_sb[:], in0=agg[:], scalar=1.0 / A_SCALE,
                    in1=node1_sb[:, g, :],
                    op0=mybir.AluOpType.mult, op1=mybir.AluOpType.add,
                )
                (nc.sync if g % 2 == 0 else nc.scalar).dma_start(
                    out=out_d[32 * g : 32 * (g + 1), :], in_=o_sb[:]
                )
                _done_groups.add(g)

            def emit_am(g):
                am = amp.tile([128, 8, 2, 32], DT8, tag="am", name=f"am{g}")
                a_src = attnT[:]
                attn_bc = bass.AP(
                    tensor=a_src.tensor,
                    offset=a_src.offset + 16 * g,
                    ap=[a_src.ap[0], [2, 8], [1, 2], [0, 32]],
                )
                nc.vector.tensor_tensor(
                    out=am[:], in0=mask_sb[:], in1=attn_bc,
                    op=mybir.AluOpType.mult,
                )
                am_tiles[g] = am


            # ---- phase 1: stream NET8 (supers), score matmuls, evac ----
            dr_kw = dict(perf_mode=DR, skip_group_check=True)
            net_tiles = {}
            for c in range(N_CHUNK):
                if c % 4 == 0:
                    sc4 = c // 4
                    net_sb4 = netp.tile([128, 4, 2, CH_SLOTS], DT8,
                                        name=f"net{sc4}", tag="net")
                    eng4 = nc.sync if sc4 % 2 == 0 else nc.scalar
                    if sc4 == 0:
                        for q in range(4):
                            eng4.dma_start(out=net_sb4[:, q],
                                           in_=net8_d[:, q, :, :])
                    else:
                        eng4.dma_start(
                            out=net_sb4[:],
                            in_=net8_d[:, 4 * sc4 : 4 * sc4 + 4, :, :],
                        )
                    net_tiles[sc4] = net_sb4
                    if sc4 == 3:
                        for SF in range(4):
                            ner_sb4 = nerp.tile([128, 4, 8, 2, D], DT8,
                                                name=f"ner{SF}", tag="ner")
                            (nc.scalar if SF % 2 == 0 else nc.sync).dma_start(
                                out=ner_sb4[:],
                                in_=ne8r_d[:, 4 * SF : 4 * SF + 4, :, :],
                            )
                            ner_tiles[SF] = ner_sb4
                net_sb = net_tiles[c // 4][:, c % 4]
                sc_ps = psum_sc.tile([128, 512], F32, tag="sc_ps")
                for m in range(4):
                    for i in range(2):
                        nc.tensor.matmul(
                            out=sc_ps[32 * m : 32 * m + 32, :],
                            lhsT=w8_sb[:, i, :],
                            rhs=net_sb[:, i, 512 * m : 512 * (m + 1)],
                            start=(i == 0),
                            stop=(i == 1),
                            tile_position=(0, 32 * m),
                            skip_group_check=True,
                        )
                scdup = scratch.tile([128, 512], DTB, name=f"scd{c}",
                                     tag="scdup")
                if c % 2 == 0:
                    nc.vector.tensor_copy(out=scdup[:], in_=sc_ps[:])
                else:
                    nc.scalar.copy(out=scdup[:], in_=sc_ps[:])
                sd = scdup[:]
                in_ap = bass.AP(
                    tensor=sd.tensor,
                    offset=sd.offset,
                    ap=[[32 * sd.ap[0][0], 4]] + list(sd.ap[1:]),
                )
                nc.gpsimd.dma_start(
                    out=scores_sb[32 * (c % 4) : 32 * (c % 4) + 4, c // 4, :],
                    in_=in_ap,
                )
                if c in (3, 5, 6):
                    emit_c_build((3, 5, 6).index(c))
                if c == 11:
                    emit_softmax(0, 2)
                elif c == 13:
                    emit_am(0)
                    emit_group(0)
                elif c == 14:
                    emit_softmax(2, 3)
                    emit_am(1)
                    emit_group(1)
                elif c == 15:
                    emit_am(2)
                    emit_group(2)
                    emit_am(3)
                    emit_group(3)


            for g in range(4, 12):
                if g not in am_tiles and g not in _done_groups:
                    emit_am(g)
                if g + 1 < 12 and g + 1 not in am_tiles \
                        and g + 1 not in _done_groups:
                    emit_am(g + 1)
                if g not in _done_groups:
                    emit_group(g)
            emit_softmax(3, 4)

            # ---- phase 2 driver ----
            for g in range(N_CHUNK):
                if g not in am_tiles and g not in _done_groups:
                    emit_am(g)
                if g + 1 < N_CHUNK and g + 1 not in am_tiles \
                        and g + 1 not in _done_groups:
                    emit_am(g + 1)
                if g not in _done_groups:
                    emit_group(g)

    nc.compile()
    return nc


def _prep_core_inputs(core, node, neighbors, emb8, emb_f32):
    """Host-side sharding: pregather this core's neighbor rows into the
    two fp8 layouts plus the fp32/bf16 node tables."""
    node_c = np.asarray(node[B_LOC * core : B_LOC * (core + 1)])
    nb = np.asarray(neighbors[:, node_c, :])          # [S, B_LOC, K]
    flat = nb.transpose(1, 0, 2).reshape(-1)          # slot = b*64 + s*32 + k
    rows8 = emb8[flat]                                # [SLOTS, D] fp8

    # NET8[p, c, i, f] = rows8[2048c + f, 128i + p]
    net8 = np.ascontiguousarray(
        rows8.view(np.uint8).reshape(N_CHUNK, CH_SLOTS, 2, 128)
        .transpose(3, 0, 2, 1)
    ).view(F8)
    # NE8R[p, c, j, i, d] = rows8[2048c + 256j + 128i + p, d]
    ne8r = np.ascontiguousarray(
        rows8.view(np.uint8).reshape(N_CHUNK, 8, 2, 128, D)
        .transpose(3, 0, 1, 2, 4)
    ).view(F8)

    nd = emb_f32[node_c]                              # [B_LOC, D] f32
    # node1[q, g, :] = 64 * node_e[32g + q]
    node1 = np.ascontiguousarray(
        (nd * float(STEPS * K)).reshape(16, 32, D).transpose(1, 0, 2)
    )
    # node2[P, m, F, s8, :] = node_e[b], b = 128F + 32P + 8m + s8  (bf16)
    P = np.arange(4)[:, None, None, None]
    m = np.arange(4)[None, :, None, None]
    F = np.arange(4)[None, None, :, None]
    s8 = np.arange(8)[None, None, None, :]
    bmap = 128 * F + 32 * P + 8 * m + s8            # [4, 4, 4, 8]
    node2 = np.ascontiguousarray(
        nd[bmap].astype(BF16).reshape(4, 4, 32 * D)
    )

    return {"net8": net8, "ne8r": ne8r, "node1": node1, "node2": node2}


def kernel(node, neighbors, embeddings, fc_w, fc_b, _trace=False):
    node = np.asarray(node)
    neighbors = np.asarray(neighbors)
    emb_f32 = np.asarray(embeddings, dtype=np.float32)
    fc_w = np.asarray(fc_w, dtype=np.float32)
    fc_b = np.asarray(fc_b, dtype=np.float32)

    key = (fc_w.tobytes(), fc_b.tobytes())
    if _CACHE.get("key") != key:
        _CACHE["nc"] = _build_nc(fc_w, fc_b)
        _CACHE["key"] = key
    nc = _CACHE["nc"]

    emb8 = emb_f32.astype(F8)
    in_maps = [
        _prep_core_inputs(c, node, neighbors, emb8, emb_f32)
        for c in range(N_CORES)
    ]
    res = run_bass_kernel_spmd(
        nc, in_maps, core_ids=list(range(N_CORES)), trace=_trace
    )
    out = np.concatenate([res.results[c]["out"] for c in range(N_CORES)], axis=0)
    if _trace:
        _CACHE["last_exec_time_ns"] = res.exec_time_ns
        _CACHE["last_results"] = res
    return out
